# revision 9
# baseline (speedup 1.0000x reference)
"""Causal self-attention (B=2, T=2048, D=2048, 16 heads) on 8 trn2 cores.

Sharding: tensor-parallel over heads - 2 heads per core. Each core computes
q/k/v projections for its 2 heads (column-parallel), causal attention per
head, and a partial output projection (row-parallel). Host sums the 8
partial outputs.

Design notes (v2, tuned for PE p-state + instruction-count):
  - warmup matmuls at t=0 ramp the PE clock while the first DMAs stream.
  - ONE pass over x computes q/k/v for BOTH heads (6 psum banks); x is
    DMA'd once per core in 8 big [128,16,512] chunks.
  - attention: per-chunk pt buffer [128, nj*512]; denominators accumulate
    on the PE (ones-matmul per j-tile); per-jt S->exp->den/PV software
    pipeline keeps both PE and ACT busy.
  - out-projection for ic-group g is emitted inside the attention stream
    of group g+1 so its matmuls fill scalar-bound bubbles; y is written
    bf16 in 32 batched DMAs and summed on host.
"""

import math
from contextlib import ExitStack

import numpy as np
import ml_dtypes

import concourse.bass as bass
import concourse.mybir as mybir
import concourse.tile as tile
from concourse import bacc
from concourse.bass_utils import run_bass_kernel_spmd
from concourse.masks import make_identity

P = 128
D_MODEL = 2048
NUM_HEADS = 16
D = 128            # head dim
B, T = 2, 2048
BT = B * T         # 4096
NCORES = 8
HPC = NUM_HEADS // NCORES   # 2 heads per core
KD = D_MODEL // P           # 16 d_model tiles
TJ = T // P                 # 16 key tiles per batch
IC = 512                    # query / token chunk width
NI = T // IC                # 4 query chunks per batch
TCH = BT // IC              # 8 token chunks for projections

F32 = mybir.dt.float32
BF16 = mybir.dt.bfloat16
NWARM = 20                  # PE p-state warmup matmuls

CFG_BF16 = dict()
CFG_SAFE = CFG_FAST = CFG_F32R = CFG_BF16


def _emit(tc, xT, wqT, wkT, wvT, woT, y):
    nc = tc.nc
    scale = 1.0 / math.sqrt(D)

    with ExitStack() as ctx:
        consts = ctx.enter_context(tc.tile_pool(name="consts", bufs=1))
        wpool = ctx.enter_context(tc.tile_pool(name="wpool", bufs=1))
        xpool = ctx.enter_context(tc.tile_pool(name="xpool", bufs=3))
        arrs = ctx.enter_context(tc.tile_pool(name="arrs", bufs=1))
        ptpool = ctx.enter_context(tc.tile_pool(name="ptpool", bufs=2))
        smalls = ctx.enter_context(tc.tile_pool(name="smalls", bufs=2))
        ypool = ctx.enter_context(tc.tile_pool(name="ypool", bufs=2))
        psum = ctx.enter_context(tc.tile_pool(name="psum", bufs=1, space="PSUM"))

        # ---- constants ----
        ident = consts.tile([P, P], BF16, tag="ident", name="ident")
        make_identity(nc, ident)
        ones_col = consts.tile([P, 1], BF16, tag="ones", name="ones")
        nc.vector.memset(ones_col, 1.0)
        # tri_mask[p, i] = 1.0 if i >= p else 0 (keep lower triangle of S)
        tri_mask = consts.tile([P, P], BF16, tag="trimask", name="trimask")
        nc.gpsimd.memset(tri_mask, 0.0)
        nc.gpsimd.affine_select(
            out=tri_mask, in_=tri_mask, compare_op=mybir.AluOpType.is_gt,
            fill=1.0, base=0, pattern=[[-1, P]], channel_multiplier=1,
        )
        wtmp = consts.tile([P, IC], BF16, tag="wtmp", name="wtmp")
        nc.vector.memset(wtmp, 0.125)

        # ---- weight DMAs (wq/wk first: needed by the first projections) ----
        w3 = {"q": wqT.rearrange("(ko p) o -> p ko o", p=P),
              "k": wkT.rearrange("(ko p) o -> p ko o", p=P),
              "v": wvT.rearrange("(ko p) o -> p ko o", p=P)}
        xT3 = xT.rearrange("(ko p) t -> p ko t", p=P)
        w_sb = {}
        for nm in ("q", "k", "v"):
            wt = wpool.tile([P, KD, HPC * D], BF16, tag=f"w{nm}", name=f"w{nm}")
            w_sb[nm] = wt
        nc.sync.dma_start(w_sb["q"], w3["q"])
        xt0 = xpool.tile([P, KD, IC], BF16, tag="xt", name="xt")
        nc.sync.dma_start(xt0, xT3[:, :, 0:IC])
        nc.sync.dma_start(w_sb["k"], w3["k"])
        nc.sync.dma_start(w_sb["v"], w3["v"])
        woT3 = woT.rearrange("(h p) m -> p h m", p=P)
        wo_sb = wpool.tile([P, HPC, D_MODEL], BF16, tag="wo", name="wo")
        nc.sync.dma_start(wo_sb, woT3)

        # ---- PE warmup: ramp p-state while DMAs stream ----
        ps_w = psum.tile([P, IC], F32, tag="pj0", name="warm")
        for _ in range(NWARM):
            nc.tensor.matmul(ps_w, wtmp[:, :P], wtmp, start=True, stop=True)

        # ---- phase A: projections, both heads, one x pass ----
        qT = [arrs.tile([P, BT], BF16, tag=f"qT{h}", name=f"qT{h}")
              for h in range(HPC)]
        kT = [arrs.tile([P, BT], BF16, tag=f"kT{h}", name=f"kT{h}")
              for h in range(HPC)]
        v_sb = [arrs.tile([P, B, TJ, D], BF16, tag=f"v{h}", name=f"v{h}")
                for h in range(HPC)]
        outT = [arrs.tile([P, BT], BF16, tag=f"o{h}", name=f"o{h}")
                for h in range(HPC)]
        vtmp = [arrs.tile([P, IC], BF16, tag=f"vt{h}", name=f"vt{h}")
                for h in range(HPC)]

        dests = [("q", 0), ("k", 0), ("q", 1), ("k", 1), ("v", 0), ("v", 1)]
        for tch in range(TCH):
            tsl = slice(tch * IC, (tch + 1) * IC)
            if tch == 0:
                xt = xt0
            else:
                xt = xpool.tile([P, KD, IC], BF16, tag="xt", name="xt")
                nc.sync.dma_start(xt, xT3[:, :, tsl])
            for di, (nm, h) in enumerate(dests):
                ps = psum.tile([P, IC], F32, tag=f"pj{di}", name=f"pj{di}")
                for kt in range(KD):
                    nc.tensor.matmul(
                        ps, w_sb[nm][:, kt, h * D:(h + 1) * D], xt[:, kt],
                        start=(kt == 0), stop=(kt == KD - 1),
                    )
                if nm == "q":
                    nc.vector.tensor_copy(qT[h][:, tsl], ps)
                elif nm == "k":
                    nc.vector.tensor_copy(kT[h][:, tsl], ps)
                else:
                    nc.vector.tensor_copy(vtmp[h], ps)
            # transpose v chunk -> v_sb (token tiles on partitions)
            b = (tch * IC) // T
            jt0 = ((tch * IC) % T) // P
            for h in range(HPC):
                pst = psum.tile([P, IC], BF16, tag="tr", name="tr")
                for q4 in range(4):
                    nc.tensor.transpose(
                        pst[:, q4 * P:(q4 + 1) * P],
                        vtmp[h][:, q4 * P:(q4 + 1) * P], ident)
                nc.vector.tensor_copy(v_sb[h][:, b, jt0:jt0 + 4], pst)

        # ---- phase B: attention (per-chunk) + interleaved out-projection ----
        def attn_chunk(ck, ic, h, b):
            i0 = b * T + ic * IC
            nj = 4 * (ic + 1)
            lo_of = lambda jt: max(jt - 4 * ic, 0) * P
            pt = ptpool.tile([P, 16 * IC], BF16, tag="pt", name="pt")

            def s_tile(jt):
                lo = lo_of(jt)
                ps_s = psum.tile([P, IC], F32, tag=f"pj{jt % 3}",
                                 name=f"pj{jt % 3}")
                nc.tensor.matmul(
                    ps_s[:, lo:],
                    kT[h][:, b * T + jt * P: b * T + (jt + 1) * P],
                    qT[h][:, i0 + lo: i0 + IC], start=True, stop=True)
                nc.scalar.activation(
                    pt[:, jt * IC + lo:(jt + 1) * IC], ps_s[:, lo:],
                    mybir.ActivationFunctionType.Exp, scale=scale)
                if jt - 4 * ic >= 0:
                    nc.vector.tensor_tensor(
                        pt[:, jt * IC + lo: jt * IC + lo + P],
                        pt[:, jt * IC + lo: jt * IC + lo + P],
                        tri_mask, mybir.AluOpType.mult)

            ps_d = psum.tile([P, IC], F32, tag="pj5", name="pj5")
            r = (ck % 3) * 32
            # PV double-buffered (pj3/pj4) so the norm chain of chunk n
            # overlaps chunk n+1's PV accumulation
            ps_o = psum.tile([P, IC], F32, tag=f"pj{3 + ck % 2}",
                             name=f"pj{3 + ck % 2}")

            # software pipeline: S runs one j-tile ahead of den/PV
            s_tile(0)
            for jt in range(nj):
                if jt + 1 < nj:
                    s_tile(jt + 1)
                lo = lo_of(jt)
                psl = slice(jt * IC + lo, (jt + 1) * IC)
                nc.tensor.matmul(
                    ps_d[r:r + 1, lo:], ones_col, pt[:, psl],
                    start=(jt == 0), stop=(jt == nj - 1),
                    skip_group_check=True)
                nc.tensor.matmul(
                    ps_o[:, lo:], v_sb[h][:, b, jt], pt[:, psl],
                    start=(jt == 0), stop=(jt == nj - 1),
                    skip_group_check=True)

            den_sb = smalls.tile([1, IC], F32, tag="densb", name="densb")
            nc.vector.tensor_copy(den_sb, ps_d[r:r + 1])
            bc = smalls.tile([P, IC], F32, tag="bc", name="bc")
            nc.gpsimd.partition_broadcast(bc, den_sb)
            rb = smalls.tile([P, IC], F32, tag="rb", name="rb")
            nc.vector.reciprocal_approx_fast(out=rb, in_=bc)
            nc.vector.tensor_tensor(
                outT[h][:, i0:i0 + IC], ps_o, rb, mybir.AluOpType.mult)

        def outproj_group(ic, b):
            # y rows for tokens of chunk (b, ic); psum->sbuf casts alternate
            # between vector and scalar so neither engine bottlenecks
            t0 = (b * T + ic * IC) // P
            for tt in range(t0, t0 + IC // P):
                y_sb = ypool.tile([P, D_MODEL], BF16, tag="ysb", name="ysb")
                for mc in range(D_MODEL // IC):
                    msl = slice(mc * IC, (mc + 1) * IC)
                    ps_y = psum.tile([P, IC], F32, tag="tr", name="yps")
                    for h in range(HPC):
                        nc.tensor.matmul(
                            ps_y, outT[h][:, tt * P:(tt + 1) * P],
                            wo_sb[:, h, msl],
                            start=(h == 0), stop=(h == HPC - 1))
                    if mc % 2 == 0:
                        nc.vector.tensor_copy(y_sb[:, msl], ps_y)
                    else:
                        nc.scalar.copy(y_sb[:, msl], ps_y)
                nc.sync.dma_start(y[tt * P:(tt + 1) * P, :], y_sb)

        ck = 0
        for ic in range(NI):
            attn_chunk(ck, ic, 0, 0); ck += 1
            if ic > 0:
                outproj_group(ic - 1, 1)
            attn_chunk(ck, ic, 1, 0); ck += 1
            attn_chunk(ck, ic, 0, 1); ck += 1
            outproj_group(ic, 0)
            attn_chunk(ck, ic, 1, 1); ck += 1
        outproj_group(NI - 1, 1)


def _build():
    nc = bacc.Bacc("TRN2", target_bir_lowering=False, debug=False,
                   num_devices=NCORES)
    xT = nc.dram_tensor("xT", [D_MODEL, BT], BF16, kind="ExternalInput").ap()
    wqT = nc.dram_tensor("wqT", [D_MODEL, HPC * D], BF16,
                         kind="ExternalInput").ap()
    wkT = nc.dram_tensor("wkT", [D_MODEL, HPC * D], BF16,
                         kind="ExternalInput").ap()
    wvT = nc.dram_tensor("wvT", [D_MODEL, HPC * D], BF16,
                         kind="ExternalInput").ap()
    woT = nc.dram_tensor("woT", [HPC * D, D_MODEL], BF16,
                         kind="ExternalInput").ap()
    y = nc.dram_tensor("y", [BT, D_MODEL], BF16, kind="ExternalOutput").ap()
    with tile.TileContext(nc) as tc:
        _emit(tc, xT, wqT, wkT, wvT, woT, y)
    nc.compile()
    return nc


def _prep_inputs(x, Wq, Wk, Wv, Wo):
    bf = ml_dtypes.bfloat16
    xT = np.ascontiguousarray(
        np.asarray(x, np.float32).reshape(BT, D_MODEL).T).astype(bf)
    in_maps = []
    for c in range(NCORES):
        rows = slice(c * HPC * D, (c + 1) * HPC * D)
        in_maps.append({
            "xT": xT,
            "wqT": np.ascontiguousarray(np.asarray(Wq)[rows].T).astype(bf),
            "wkT": np.ascontiguousarray(np.asarray(Wk)[rows].T).astype(bf),
            "wvT": np.ascontiguousarray(np.asarray(Wv)[rows].T).astype(bf),
            "woT": np.ascontiguousarray(np.asarray(Wo)[:, rows].T).astype(bf),
        })
    return in_maps


def run(x, Wq, Wk, Wv, Wo, cfg=None, trace=False):
    nc = _build()
    in_maps = _prep_inputs(x, Wq, Wk, Wv, Wo)
    try:
        res = run_bass_kernel_spmd(nc, in_maps, core_ids=list(range(NCORES)),
                                   trace=trace)
    except Exception:
        res = run_bass_kernel_spmd(nc, in_maps, core_ids=list(range(NCORES)),
                                   trace=trace)
    y = np.zeros((BT, D_MODEL), np.float32)
    for r in res.results:
        y += np.asarray(r["y"], dtype=np.float32)
    return y.reshape(B, T, D_MODEL), res


def kernel(x, Wq, Wk, Wv, Wo):
    y, _ = run(x, Wq, Wk, Wv, Wo)
    return y


# revision 12
# speedup vs baseline: 1.0521x; 1.0521x over previous
"""Causal self-attention (B=2, T=2048, D=2048, 16 heads) on 8 trn2 cores.

Sharding: tensor-parallel over heads - 2 heads per core. Each core computes
q/k/v projections for its 2 heads (column-parallel), causal attention per
head, and a partial output projection (row-parallel). Host sums the 8
partial outputs.

Design notes (v2, tuned for PE p-state + instruction-count):
  - warmup matmuls at t=0 ramp the PE clock while the first DMAs stream.
  - ONE pass over x computes q/k/v for BOTH heads (6 psum banks); x is
    DMA'd once per core in 8 big [128,16,512] chunks.
  - attention: per-chunk pt buffer [128, nj*512]; denominators accumulate
    on the PE (ones-matmul per j-tile); per-jt S->exp->den/PV software
    pipeline keeps both PE and ACT busy.
  - out-projection for ic-group g is emitted inside the attention stream
    of group g+1 so its matmuls fill scalar-bound bubbles; y is written
    bf16 in 32 batched DMAs and summed on host.
"""

import math
from contextlib import ExitStack

import numpy as np
import ml_dtypes

import concourse.bass as bass
import concourse.mybir as mybir
import concourse.tile as tile
from concourse import bacc
from concourse.bass_utils import run_bass_kernel_spmd
from concourse.masks import make_identity

P = 128
D_MODEL = 2048
NUM_HEADS = 16
D = 128            # head dim
B, T = 2, 2048
BT = B * T         # 4096
NCORES = 8
HPC = NUM_HEADS // NCORES   # 2 heads per core
KD = D_MODEL // P           # 16 d_model tiles
TJ = T // P                 # 16 key tiles per batch
IC = 512                    # query / token chunk width
NI = T // IC                # 4 query chunks per batch
TCH = BT // IC              # 8 token chunks for projections

F32 = mybir.dt.float32
BF16 = mybir.dt.bfloat16
NWARM = 26                  # PE p-state warmup matmuls

CFG_BF16 = dict()
CFG_SAFE = CFG_FAST = CFG_F32R = CFG_BF16


def _emit(tc, xT, wqT, wkT, wvT, woT, y):
    nc = tc.nc
    scale = 1.0 / math.sqrt(D)

    with ExitStack() as ctx:
        consts = ctx.enter_context(tc.tile_pool(name="consts", bufs=1))
        wpool = ctx.enter_context(tc.tile_pool(name="wpool", bufs=1))
        xpool = ctx.enter_context(tc.tile_pool(name="xpool", bufs=3))
        arrs = ctx.enter_context(tc.tile_pool(name="arrs", bufs=1))
        ptpool = ctx.enter_context(tc.tile_pool(name="ptpool", bufs=2))
        smalls = ctx.enter_context(tc.tile_pool(name="smalls", bufs=2))
        ypool = ctx.enter_context(tc.tile_pool(name="ypool", bufs=2))
        psum = ctx.enter_context(tc.tile_pool(name="psum", bufs=1, space="PSUM"))

        # ---- constants ----
        ident = consts.tile([P, P], BF16, tag="ident", name="ident")
        make_identity(nc, ident)
        ones_col = consts.tile([P, 1], BF16, tag="ones", name="ones")
        nc.vector.memset(ones_col, 1.0)
        # tri_mask[p, i] = 1.0 if i >= p else 0 (keep lower triangle of S)
        tri_mask = consts.tile([P, P], BF16, tag="trimask", name="trimask")
        nc.gpsimd.memset(tri_mask, 0.0)
        nc.gpsimd.affine_select(
            out=tri_mask, in_=tri_mask, compare_op=mybir.AluOpType.is_gt,
            fill=1.0, base=0, pattern=[[-1, P]], channel_multiplier=1,
        )
        wtmp = consts.tile([P, IC], BF16, tag="wtmp", name="wtmp")
        nc.vector.memset(wtmp, 0.125)

        # ---- weight DMAs (wq/wk first: needed by the first projections) ----
        w3 = {"q": wqT.rearrange("(ko p) o -> p ko o", p=P),
              "k": wkT.rearrange("(ko p) o -> p ko o", p=P),
              "v": wvT.rearrange("(ko p) o -> p ko o", p=P)}
        xT3 = xT.rearrange("(ko p) t -> p ko t", p=P)
        w_sb = {}
        for nm in ("q", "k", "v"):
            wt = wpool.tile([P, KD, HPC * D], BF16, tag=f"w{nm}", name=f"w{nm}")
            w_sb[nm] = wt
        nc.sync.dma_start(w_sb["q"], w3["q"])
        xt0 = xpool.tile([P, KD, IC], BF16, tag="xt", name="xt")
        nc.sync.dma_start(xt0, xT3[:, :, 0:IC])
        nc.sync.dma_start(w_sb["k"], w3["k"])
        nc.sync.dma_start(w_sb["v"], w3["v"])
        woT3 = woT.rearrange("(h p) m -> p h m", p=P)
        wo_sb = wpool.tile([P, HPC, D_MODEL], BF16, tag="wo", name="wo")
        nc.sync.dma_start(wo_sb, woT3)

        # ---- PE warmup: ramp p-state while DMAs stream ----
        ps_w = psum.tile([P, IC], F32, tag="pj0", name="warm")
        for _ in range(NWARM):
            nc.tensor.matmul(ps_w, wtmp[:, :P], wtmp, start=True, stop=True)

        # ---- phase A: projections, both heads, one x pass ----
        qT = [arrs.tile([P, BT], BF16, tag=f"qT{h}", name=f"qT{h}")
              for h in range(HPC)]
        kT = [arrs.tile([P, BT], BF16, tag=f"kT{h}", name=f"kT{h}")
              for h in range(HPC)]
        v_sb = [arrs.tile([P, B, TJ, D], BF16, tag=f"v{h}", name=f"v{h}")
                for h in range(HPC)]
        outT = [arrs.tile([P, BT], BF16, tag=f"o{h}", name=f"o{h}")
                for h in range(HPC)]
        vtmp = [arrs.tile([P, IC], BF16, tag=f"vt{h}", name=f"vt{h}")
                for h in range(HPC)]

        dests = [("q", 0), ("k", 0), ("q", 1), ("k", 1), ("v", 0), ("v", 1)]
        for tch in range(TCH):
            tsl = slice(tch * IC, (tch + 1) * IC)
            if tch == 0:
                xt = xt0
            else:
                xt = xpool.tile([P, KD, IC], BF16, tag="xt", name="xt")
                nc.sync.dma_start(xt, xT3[:, :, tsl])
            for di, (nm, h) in enumerate(dests):
                ps = psum.tile([P, IC], F32, tag=f"pj{di}", name=f"pj{di}")
                for kt in range(KD):
                    nc.tensor.matmul(
                        ps, w_sb[nm][:, kt, h * D:(h + 1) * D], xt[:, kt],
                        start=(kt == 0), stop=(kt == KD - 1),
                    )
                if nm == "q":
                    nc.vector.tensor_copy(qT[h][:, tsl], ps)
                elif nm == "k":
                    nc.vector.tensor_copy(kT[h][:, tsl], ps)
                else:
                    nc.vector.tensor_copy(vtmp[h], ps)
            # transpose v chunk -> v_sb (token tiles on partitions)
            b = (tch * IC) // T
            jt0 = ((tch * IC) % T) // P
            for h in range(HPC):
                pst = psum.tile([P, IC], BF16, tag="tr", name="tr")
                for q4 in range(4):
                    nc.tensor.transpose(
                        pst[:, q4 * P:(q4 + 1) * P],
                        vtmp[h][:, q4 * P:(q4 + 1) * P], ident)
                nc.vector.tensor_copy(v_sb[h][:, b, jt0:jt0 + 4], pst)

        # ---- phase B: attention (per-chunk) + interleaved out-projection ----
        def attn_chunk(ck, ic, h, b):
            i0 = b * T + ic * IC
            nj = 4 * (ic + 1)
            lo_of = lambda jt: max(jt - 4 * ic, 0) * P
            pt = ptpool.tile([P, 16 * IC], BF16, tag="pt", name="pt")

            def s_tile(jt):
                lo = lo_of(jt)
                ps_s = psum.tile([P, IC], F32, tag=f"pj{jt % 3}",
                                 name=f"pj{jt % 3}")
                nc.tensor.matmul(
                    ps_s[:, lo:],
                    kT[h][:, b * T + jt * P: b * T + (jt + 1) * P],
                    qT[h][:, i0 + lo: i0 + IC], start=True, stop=True)
                nc.scalar.activation(
                    pt[:, jt * IC + lo:(jt + 1) * IC], ps_s[:, lo:],
                    mybir.ActivationFunctionType.Exp, scale=scale)
                if jt - 4 * ic >= 0:
                    nc.vector.tensor_tensor(
                        pt[:, jt * IC + lo: jt * IC + lo + P],
                        pt[:, jt * IC + lo: jt * IC + lo + P],
                        tri_mask, mybir.AluOpType.mult)

            ps_d = psum.tile([P, IC], F32, tag="pj5", name="pj5")
            r = (ck % 3) * 32
            # PV double-buffered (pj3/pj4) so the norm chain of chunk n
            # overlaps chunk n+1's PV accumulation
            ps_o = psum.tile([P, IC], F32, tag=f"pj{3 + ck % 2}",
                             name=f"pj{3 + ck % 2}")

            # software pipeline: S runs one j-tile ahead of den/PV
            s_tile(0)
            for jt in range(nj):
                if jt + 1 < nj:
                    s_tile(jt + 1)
                lo = lo_of(jt)
                psl = slice(jt * IC + lo, (jt + 1) * IC)
                nc.tensor.matmul(
                    ps_d[r:r + 1, lo:], ones_col, pt[:, psl],
                    start=(jt == 0), stop=(jt == nj - 1),
                    skip_group_check=True)
                nc.tensor.matmul(
                    ps_o[:, lo:], v_sb[h][:, b, jt], pt[:, psl],
                    start=(jt == 0), stop=(jt == nj - 1),
                    skip_group_check=True)

            den_sb = smalls.tile([1, IC], F32, tag="densb", name="densb")
            nc.vector.tensor_copy(den_sb, ps_d[r:r + 1])
            bc = smalls.tile([P, IC], F32, tag="bc", name="bc")
            nc.gpsimd.partition_broadcast(bc, den_sb)
            rb = smalls.tile([P, IC], F32, tag="rb", name="rb")
            nc.vector.reciprocal_approx_fast(out=rb, in_=bc)
            nc.vector.tensor_tensor(
                outT[h][:, i0:i0 + IC], ps_o, rb, mybir.AluOpType.mult)

        def outproj_group(ic, b, deep=False):
            # y rows for tokens of chunk (b, ic); psum->sbuf casts alternate
            # between vector and scalar so neither engine bottlenecks.
            # deep=True rotates psum across 5 buffers (attention banks are
            # free) so matmul pairs never wait on casts.
            t0 = (b * T + ic * IC) // P
            u = 0
            for tt in range(t0, t0 + IC // P):
                y_sb = ypool.tile([P, D_MODEL], BF16, tag="ysb", name="ysb")
                for mc in range(D_MODEL // IC):
                    msl = slice(mc * IC, (mc + 1) * IC)
                    ytag = ["tr", "pj0", "pj1", "pj2"][u % 4] if deep else "tr"
                    ps_y = psum.tile([P, IC], F32, tag=ytag, name="yps")
                    u += 1
                    for h in range(HPC):
                        nc.tensor.matmul(
                            ps_y, outT[h][:, tt * P:(tt + 1) * P],
                            wo_sb[:, h, msl],
                            start=(h == 0), stop=(h == HPC - 1))
                    if mc % 2 == 0:
                        nc.vector.tensor_copy(y_sb[:, msl], ps_y)
                    else:
                        nc.scalar.copy(y_sb[:, msl], ps_y)
                nc.sync.dma_start(y[tt * P:(tt + 1) * P, :], y_sb)

        ck = 0
        for ic in range(NI):
            attn_chunk(ck, ic, 0, 0); ck += 1
            if ic > 0:
                outproj_group(ic - 1, 1)
            attn_chunk(ck, ic, 1, 0); ck += 1
            attn_chunk(ck, ic, 0, 1); ck += 1
            outproj_group(ic, 0, deep=(ic == NI - 1))
            attn_chunk(ck, ic, 1, 1); ck += 1
        outproj_group(NI - 1, 1, deep=True)


def _build():
    nc = bacc.Bacc("TRN2", target_bir_lowering=False, debug=False,
                   num_devices=NCORES)
    xT = nc.dram_tensor("xT", [D_MODEL, BT], BF16, kind="ExternalInput").ap()
    wqT = nc.dram_tensor("wqT", [D_MODEL, HPC * D], BF16,
                         kind="ExternalInput").ap()
    wkT = nc.dram_tensor("wkT", [D_MODEL, HPC * D], BF16,
                         kind="ExternalInput").ap()
    wvT = nc.dram_tensor("wvT", [D_MODEL, HPC * D], BF16,
                         kind="ExternalInput").ap()
    woT = nc.dram_tensor("woT", [HPC * D, D_MODEL], BF16,
                         kind="ExternalInput").ap()
    y = nc.dram_tensor("y", [BT, D_MODEL], BF16, kind="ExternalOutput").ap()
    with tile.TileContext(nc) as tc:
        _emit(tc, xT, wqT, wkT, wvT, woT, y)
    nc.compile()
    return nc


def _prep_inputs(x, Wq, Wk, Wv, Wo):
    bf = ml_dtypes.bfloat16
    xT = np.ascontiguousarray(
        np.asarray(x, np.float32).reshape(BT, D_MODEL).T).astype(bf)
    in_maps = []
    for c in range(NCORES):
        rows = slice(c * HPC * D, (c + 1) * HPC * D)
        in_maps.append({
            "xT": xT,
            "wqT": np.ascontiguousarray(np.asarray(Wq)[rows].T).astype(bf),
            "wkT": np.ascontiguousarray(np.asarray(Wk)[rows].T).astype(bf),
            "wvT": np.ascontiguousarray(np.asarray(Wv)[rows].T).astype(bf),
            "woT": np.ascontiguousarray(np.asarray(Wo)[:, rows].T).astype(bf),
        })
    return in_maps


def run(x, Wq, Wk, Wv, Wo, cfg=None, trace=False):
    nc = _build()
    in_maps = _prep_inputs(x, Wq, Wk, Wv, Wo)
    try:
        res = run_bass_kernel_spmd(nc, in_maps, core_ids=list(range(NCORES)),
                                   trace=trace)
    except Exception:
        res = run_bass_kernel_spmd(nc, in_maps, core_ids=list(range(NCORES)),
                                   trace=trace)
    y = np.zeros((BT, D_MODEL), np.float32)
    for r in res.results:
        y += np.asarray(r["y"], dtype=np.float32)
    return y.reshape(B, T, D_MODEL), res


def kernel(x, Wq, Wk, Wv, Wo):
    y, _ = run(x, Wq, Wk, Wv, Wo)
    return y


# revision 18
# speedup vs baseline: 1.1016x; 1.0471x over previous
"""Causal self-attention (B=2, T=2048, D=2048, 16 heads) on 8 trn2 cores.

Sharding: tensor-parallel over heads - 2 heads per core. Each core computes
q/k/v projections for its 2 heads (column-parallel), causal attention per
head, and a partial output projection (row-parallel). Host sums the 8
partial outputs.

Design notes (v2, tuned for PE p-state + instruction-count):
  - warmup matmuls at t=0 ramp the PE clock while the first DMAs stream.
  - ONE pass over x computes q/k/v for BOTH heads (6 psum banks); x is
    DMA'd once per core in 8 big [128,16,512] chunks.
  - attention: per-chunk pt buffer [128, nj*512]; denominators accumulate
    on the PE (ones-matmul per j-tile); per-jt S->exp->den/PV software
    pipeline keeps both PE and ACT busy.
  - out-projection for ic-group g is emitted inside the attention stream
    of group g+1 so its matmuls fill scalar-bound bubbles; y is written
    bf16 in 32 batched DMAs and summed on host.
"""

import math
from contextlib import ExitStack

import numpy as np
import ml_dtypes

import concourse.bass as bass
import concourse.mybir as mybir
import concourse.tile as tile
from concourse import bacc
from concourse.bass_utils import run_bass_kernel_spmd
from concourse.masks import make_identity

P = 128
D_MODEL = 2048
NUM_HEADS = 16
D = 128            # head dim
B, T = 2, 2048
BT = B * T         # 4096
NCORES = 8
HPC = NUM_HEADS // NCORES   # 2 heads per core
KD = D_MODEL // P           # 16 d_model tiles
TJ = T // P                 # 16 key tiles per batch
IC = 512                    # query / token chunk width
NI = T // IC                # 4 query chunks per batch
TCH = BT // IC              # 8 token chunks for projections

F32 = mybir.dt.float32
BF16 = mybir.dt.bfloat16
NWARM = 14                  # PE p-state warmup matmuls

CFG_BF16 = dict()
CFG_SAFE = CFG_FAST = CFG_F32R = CFG_BF16


def _emit(tc, xT, wqT, wkT, wvT, woT, y):
    nc = tc.nc
    scale = 1.0 / math.sqrt(D)

    with ExitStack() as ctx:
        consts = ctx.enter_context(tc.tile_pool(name="consts", bufs=1))
        wpool = ctx.enter_context(tc.tile_pool(name="wpool", bufs=1))
        xpool = ctx.enter_context(tc.tile_pool(name="xpool", bufs=3))
        arrs = ctx.enter_context(tc.tile_pool(name="arrs", bufs=1))
        ptpool = ctx.enter_context(tc.tile_pool(name="ptpool", bufs=2))
        smalls = ctx.enter_context(tc.tile_pool(name="smalls", bufs=2))
        ypool = ctx.enter_context(tc.tile_pool(name="ypool", bufs=2))
        psum = ctx.enter_context(tc.tile_pool(name="psum", bufs=1, space="PSUM"))

        # ---- constants ----
        ident = consts.tile([P, P], BF16, tag="ident", name="ident")
        make_identity(nc, ident)
        ones_col = consts.tile([P, 1], BF16, tag="ones", name="ones")
        nc.vector.memset(ones_col, 1.0)
        # tri_mask[p, i] = 1.0 if i >= p else 0 (keep lower triangle of S)
        tri_mask = consts.tile([P, P], BF16, tag="trimask", name="trimask")
        nc.gpsimd.memset(tri_mask, 0.0)
        nc.gpsimd.affine_select(
            out=tri_mask, in_=tri_mask, compare_op=mybir.AluOpType.is_gt,
            fill=1.0, base=0, pattern=[[-1, P]], channel_multiplier=1,
        )
        wtmp = consts.tile([P, IC], BF16, tag="wtmp", name="wtmp")
        nc.vector.memset(wtmp, 0.125)

        # ---- weight DMAs (host pre-tiled: fully contiguous per partition) ----
        w_sb = {}
        for nm, src in (("q", wqT), ("k", wkT), ("v", wvT)):
            wt = wpool.tile([P, KD, HPC * D], BF16, tag=f"w{nm}", name=f"w{nm}")
            w_sb[nm] = wt
        nc.sync.dma_start(w_sb["q"], wqT.rearrange("p (ko o) -> p ko o", ko=KD))
        xt0 = xpool.tile([P, KD, IC], BF16, tag="xt", name="xt")
        nc.sync.dma_start(xt0, xT[0:P].rearrange("p (ko t) -> p ko t", ko=KD))
        nc.sync.dma_start(w_sb["k"], wkT.rearrange("p (ko o) -> p ko o", ko=KD))
        nc.sync.dma_start(w_sb["v"], wvT.rearrange("p (ko o) -> p ko o", ko=KD))
        wo_sb = wpool.tile([P, HPC, D_MODEL], BF16, tag="wo", name="wo")
        nc.sync.dma_start(wo_sb, woT.rearrange("p (h m) -> p h m", h=HPC))

        # ---- PE warmup: ramp p-state while DMAs stream ----
        ps_w = psum.tile([P, IC], F32, tag="pj0", name="warm")
        for _ in range(NWARM):
            nc.tensor.matmul(ps_w, wtmp[:, :P], wtmp, start=True, stop=True)

        # ---- phase A: projections, both heads, one x pass ----
        qT = [arrs.tile([P, BT], BF16, tag=f"qT{h}", name=f"qT{h}")
              for h in range(HPC)]
        kT = [arrs.tile([P, BT], BF16, tag=f"kT{h}", name=f"kT{h}")
              for h in range(HPC)]
        v_sb = [arrs.tile([P, B, TJ, D], BF16, tag=f"v{h}", name=f"v{h}")
                for h in range(HPC)]
        outT = [arrs.tile([P, BT], BF16, tag=f"o{h}", name=f"o{h}")
                for h in range(HPC)]
        vtmp = [arrs.tile([P, IC], BF16, tag=f"vt{h}", name=f"vt{h}")
                for h in range(HPC)]

        dests = [("q", 0), ("k", 0), ("q", 1), ("k", 1), ("v", 0), ("v", 1)]
        for tch in range(TCH):
            tsl = slice(tch * IC, (tch + 1) * IC)
            if tch == 0:
                xt = xt0
            else:
                xt = xpool.tile([P, KD, IC], BF16, tag="xt", name="xt")
                nc.sync.dma_start(
                    xt, xT[tch * P:(tch + 1) * P].rearrange(
                        "p (ko t) -> p ko t", ko=KD))
            for di, (nm, h) in enumerate(dests):
                ps = psum.tile([P, IC], F32, tag=f"pj{di}", name=f"pj{di}")
                for kt in range(KD):
                    nc.tensor.matmul(
                        ps, w_sb[nm][:, kt, h * D:(h + 1) * D], xt[:, kt],
                        start=(kt == 0), stop=(kt == KD - 1),
                    )
                if nm == "q":
                    nc.vector.tensor_copy(qT[h][:, tsl], ps)
                elif nm == "k":
                    nc.vector.tensor_copy(kT[h][:, tsl], ps)
                else:
                    nc.vector.tensor_copy(vtmp[h], ps)
            # transpose v chunk -> v_sb (token tiles on partitions)
            b = (tch * IC) // T
            jt0 = ((tch * IC) % T) // P
            for h in range(HPC):
                pst = psum.tile([P, IC], BF16, tag="tr", name="tr")
                for q4 in range(4):
                    nc.tensor.transpose(
                        pst[:, q4 * P:(q4 + 1) * P],
                        vtmp[h][:, q4 * P:(q4 + 1) * P], ident)
                nc.vector.tensor_copy(v_sb[h][:, b, jt0:jt0 + 4], pst)

        # ---- phase B: attention (per-chunk) + interleaved out-projection ----
        def attn_chunk(ck, ic, h, b):
            i0 = b * T + ic * IC
            nj = 4 * (ic + 1)
            lo_of = lambda jt: max(jt - 4 * ic, 0) * P
            pt = ptpool.tile([P, 16 * IC], BF16, tag="pt", name="pt")

            def s_tile(jt):
                lo = lo_of(jt)
                ps_s = psum.tile([P, IC], F32, tag=f"pj{jt % 3}",
                                 name=f"pj{jt % 3}")
                nc.tensor.matmul(
                    ps_s[:, lo:],
                    kT[h][:, b * T + jt * P: b * T + (jt + 1) * P],
                    qT[h][:, i0 + lo: i0 + IC], start=True, stop=True)
                nc.scalar.activation(
                    pt[:, jt * IC + lo:(jt + 1) * IC], ps_s[:, lo:],
                    mybir.ActivationFunctionType.Exp, scale=scale)
                if jt - 4 * ic >= 0:
                    nc.vector.tensor_tensor(
                        pt[:, jt * IC + lo: jt * IC + lo + P],
                        pt[:, jt * IC + lo: jt * IC + lo + P],
                        tri_mask, mybir.AluOpType.mult)

            ps_d = psum.tile([P, IC], F32, tag="pj5", name="pj5")
            r = (ck % 3) * 32
            # PV double-buffered (pj3/pj4) so the norm chain of chunk n
            # overlaps chunk n+1's PV accumulation
            ps_o = psum.tile([P, IC], F32, tag=f"pj{3 + ck % 2}",
                             name=f"pj{3 + ck % 2}")

            # software pipeline: S runs one j-tile ahead of den/PV
            s_tile(0)
            for jt in range(nj):
                if jt + 1 < nj:
                    s_tile(jt + 1)
                lo = lo_of(jt)
                psl = slice(jt * IC + lo, (jt + 1) * IC)
                nc.tensor.matmul(
                    ps_d[r:r + 1, lo:], ones_col, pt[:, psl],
                    start=(jt == 0), stop=(jt == nj - 1),
                    skip_group_check=True)
                nc.tensor.matmul(
                    ps_o[:, lo:], v_sb[h][:, b, jt], pt[:, psl],
                    start=(jt == 0), stop=(jt == nj - 1),
                    skip_group_check=True)

            den_sb = smalls.tile([1, IC], F32, tag="densb", name="densb")
            nc.vector.tensor_copy(den_sb, ps_d[r:r + 1])
            bc = smalls.tile([P, IC], F32, tag="bc", name="bc")
            nc.gpsimd.partition_broadcast(bc, den_sb)
            rb = smalls.tile([P, IC], F32, tag="rb", name="rb")
            nc.vector.reciprocal_approx_fast(out=rb, in_=bc)
            nc.vector.tensor_tensor(
                outT[h][:, i0:i0 + IC], ps_o, rb, mybir.AluOpType.mult)

        def outproj_group(ic, b, deep=False):
            # y rows for tokens of chunk (b, ic); psum->sbuf casts alternate
            # between vector and scalar so neither engine bottlenecks.
            # deep=True rotates psum across 5 buffers (attention banks are
            # free) so matmul pairs never wait on casts.
            t0 = (b * T + ic * IC) // P
            u = 0
            for tt in range(t0, t0 + IC // P):
                y_sb = ypool.tile([P, D_MODEL], BF16, tag="ysb", name="ysb")
                for mc in range(D_MODEL // IC):
                    msl = slice(mc * IC, (mc + 1) * IC)
                    ytag = ["tr", "pj0", "pj1", "pj2"][u % 4] if deep else "tr"
                    ps_y = psum.tile([P, IC], F32, tag=ytag, name="yps")
                    u += 1
                    for h in range(HPC):
                        nc.tensor.matmul(
                            ps_y, outT[h][:, tt * P:(tt + 1) * P],
                            wo_sb[:, h, msl],
                            start=(h == 0), stop=(h == HPC - 1))
                    if mc % 2 == 0:
                        nc.vector.tensor_copy(y_sb[:, msl], ps_y)
                    else:
                        nc.scalar.copy(y_sb[:, msl], ps_y)
                nc.sync.dma_start(y[tt * P:(tt + 1) * P, :], y_sb)

        ck = 0
        for ic in range(NI):
            attn_chunk(ck, ic, 0, 0); ck += 1
            if ic > 0:
                outproj_group(ic - 1, 1, deep=True)
            attn_chunk(ck, ic, 1, 0); ck += 1
            attn_chunk(ck, ic, 0, 1); ck += 1
            outproj_group(ic, 0, deep=True)
            attn_chunk(ck, ic, 1, 1); ck += 1
        outproj_group(NI - 1, 1, deep=True)


def _build():
    nc = bacc.Bacc("TRN2", target_bir_lowering=False, debug=False,
                   num_devices=NCORES)
    # host pre-tiles everything so each DMA is contiguous per partition
    xT = nc.dram_tensor("xT", [TCH * P, KD * IC], BF16,
                        kind="ExternalInput").ap()
    wqT = nc.dram_tensor("wqT", [P, KD * HPC * D], BF16,
                         kind="ExternalInput").ap()
    wkT = nc.dram_tensor("wkT", [P, KD * HPC * D], BF16,
                         kind="ExternalInput").ap()
    wvT = nc.dram_tensor("wvT", [P, KD * HPC * D], BF16,
                         kind="ExternalInput").ap()
    woT = nc.dram_tensor("woT", [P, HPC * D_MODEL], BF16,
                         kind="ExternalInput").ap()
    y = nc.dram_tensor("y", [BT, D_MODEL], BF16, kind="ExternalOutput").ap()
    with tile.TileContext(nc) as tc:
        _emit(tc, xT, wqT, wkT, wvT, woT, y)
    nc.compile()
    return nc


def _prep_inputs(x, Wq, Wk, Wv, Wo):
    bf = ml_dtypes.bfloat16

    def wtile(w):  # [D_MODEL, HPC*D] -> [P, KD*(HPC*D)], contiguous rows
        return np.ascontiguousarray(
            w.reshape(KD, P, HPC * D).transpose(1, 0, 2).reshape(P, -1)
        ).astype(bf)

    xT2 = np.asarray(x, np.float32).reshape(BT, D_MODEL).T  # [D_MODEL, BT]
    xT = np.ascontiguousarray(
        xT2.reshape(KD, P, TCH, IC).transpose(2, 1, 0, 3).reshape(TCH * P, -1)
    ).astype(bf)
    in_maps = []
    for c in range(NCORES):
        rows = slice(c * HPC * D, (c + 1) * HPC * D)
        woT2 = np.asarray(Wo)[:, rows].T  # [HPC*D, D_MODEL]
        in_maps.append({
            "xT": xT,
            "wqT": wtile(np.asarray(Wq)[rows].T),
            "wkT": wtile(np.asarray(Wk)[rows].T),
            "wvT": wtile(np.asarray(Wv)[rows].T),
            "woT": np.ascontiguousarray(
                woT2.reshape(HPC, P, D_MODEL).transpose(1, 0, 2)
                .reshape(P, -1)).astype(bf),
        })
    return in_maps


def run(x, Wq, Wk, Wv, Wo, cfg=None, trace=False):
    nc = _build()
    in_maps = _prep_inputs(x, Wq, Wk, Wv, Wo)
    try:
        res = run_bass_kernel_spmd(nc, in_maps, core_ids=list(range(NCORES)),
                                   trace=trace)
    except Exception:
        res = run_bass_kernel_spmd(nc, in_maps, core_ids=list(range(NCORES)),
                                   trace=trace)
    y = np.zeros((BT, D_MODEL), np.float32)
    for r in res.results:
        y += np.asarray(r["y"], dtype=np.float32)
    return y.reshape(B, T, D_MODEL), res


def kernel(x, Wq, Wk, Wv, Wo):
    y, _ = run(x, Wq, Wk, Wv, Wo)
    return y


# revision 25
# speedup vs baseline: 1.2169x; 1.1047x over previous
"""Causal self-attention (B=2, T=2048, D=2048, 16 heads) on 8 trn2 cores.

Sharding: tensor-parallel over heads - 2 heads per core. Each core computes
q/k/v projections for its 2 heads (column-parallel), causal attention per
head, and a partial output projection (row-parallel). Host sums the 8
partial outputs.

Design notes (v2, tuned for PE p-state + instruction-count):
  - warmup matmuls at t=0 ramp the PE clock while the first DMAs stream.
  - ONE pass over x computes q/k/v for BOTH heads (6 psum banks); x is
    DMA'd once per core in 8 big [128,16,512] chunks.
  - attention: per-chunk pt buffer [128, nj*512]; denominators accumulate
    on the PE (ones-matmul per j-tile); per-jt S->exp->den/PV software
    pipeline keeps both PE and ACT busy.
  - out-projection for ic-group g is emitted inside the attention stream
    of group g+1 so its matmuls fill scalar-bound bubbles; y is written
    bf16 in 32 batched DMAs and summed on host.
"""

import math
from contextlib import ExitStack

import numpy as np
import ml_dtypes

import concourse.bass as bass
import concourse.mybir as mybir
import concourse.tile as tile
from concourse import bacc
from concourse.bass_utils import run_bass_kernel_spmd
from concourse.masks import make_identity

P = 128
D_MODEL = 2048
NUM_HEADS = 16
D = 128            # head dim
B, T = 2, 2048
BT = B * T         # 4096
NCORES = 8
HPC = NUM_HEADS // NCORES   # 2 heads per core
KD = D_MODEL // P           # 16 d_model tiles
TJ = T // P                 # 16 key tiles per batch
IC = 512                    # query / token chunk width
NI = T // IC                # 4 query chunks per batch
TCH = BT // IC              # 8 token chunks for projections

F32 = mybir.dt.float32
BF16 = mybir.dt.bfloat16
NWARM = 28                  # PE p-state warmup matmuls

CFG_BF16 = dict()
CFG_SAFE = CFG_FAST = CFG_F32R = CFG_BF16


def _emit(tc, xT, wqT, wkT, wvT, woT, y):
    nc = tc.nc
    scale = 1.0 / math.sqrt(D)

    with ExitStack() as ctx:
        consts = ctx.enter_context(tc.tile_pool(name="consts", bufs=1))
        wpool = ctx.enter_context(tc.tile_pool(name="wpool", bufs=1))
        xpool = ctx.enter_context(tc.tile_pool(name="xpool", bufs=3))
        arrs = ctx.enter_context(tc.tile_pool(name="arrs", bufs=1))
        ptpool = ctx.enter_context(tc.tile_pool(name="ptpool", bufs=2))
        smalls = ctx.enter_context(tc.tile_pool(name="smalls", bufs=2))
        ypool = ctx.enter_context(tc.tile_pool(name="ypool", bufs=2))
        psum = ctx.enter_context(tc.tile_pool(name="psum", bufs=1, space="PSUM"))

        # ---- constants ----
        ident = consts.tile([P, P], BF16, tag="ident", name="ident")
        make_identity(nc, ident)
        ones_col = consts.tile([P, 1], BF16, tag="ones", name="ones")
        nc.vector.memset(ones_col, 1.0)
        # tri_mask[p, i] = 1.0 if i >= p else 0 (keep lower triangle of S)
        tri_mask = consts.tile([P, P], BF16, tag="trimask", name="trimask")
        nc.gpsimd.memset(tri_mask, 0.0)
        nc.gpsimd.affine_select(
            out=tri_mask, in_=tri_mask, compare_op=mybir.AluOpType.is_gt,
            fill=1.0, base=0, pattern=[[-1, P]], channel_multiplier=1,
        )
        wtmp = consts.tile([P, IC], BF16, tag="wtmp", name="wtmp")
        nc.vector.memset(wtmp, 0.125)

        # ---- weight DMAs (host pre-tiled: fully contiguous per partition) ----
        w_sb = {}
        for nm, src in (("q", wqT), ("k", wkT), ("v", wvT)):
            wt = wpool.tile([P, KD, HPC * D], BF16, tag=f"w{nm}", name=f"w{nm}")
            w_sb[nm] = wt
        nc.sync.dma_start(w_sb["q"], wqT.rearrange("p (ko o) -> p ko o", ko=KD))
        xt0 = xpool.tile([P, KD, IC], BF16, tag="xt", name="xt")
        nc.sync.dma_start(xt0, xT[0:P].rearrange("p (ko t) -> p ko t", ko=KD))
        nc.sync.dma_start(w_sb["k"], wkT.rearrange("p (ko o) -> p ko o", ko=KD))
        nc.sync.dma_start(w_sb["v"], wvT.rearrange("p (ko o) -> p ko o", ko=KD))
        wo_sb = wpool.tile([P, HPC, D_MODEL], BF16, tag="wo", name="wo")
        nc.sync.dma_start(wo_sb, woT.rearrange("p (h m) -> p h m", h=HPC))

        # ---- PE warmup: ramp p-state while DMAs stream ----
        ps_w = psum.tile([P, IC], F32, tag="pj0", name="warm")
        for _ in range(NWARM):
            nc.tensor.matmul(ps_w, wtmp[:, :P], wtmp, start=True, stop=True)

        # ---- phase A: projections, both heads, one x pass ----
        qT = [arrs.tile([P, BT], BF16, tag=f"qT{h}", name=f"qT{h}")
              for h in range(HPC)]
        kT = [arrs.tile([P, BT], BF16, tag=f"kT{h}", name=f"kT{h}")
              for h in range(HPC)]
        v_sb = [arrs.tile([P, B, TJ, D], BF16, tag=f"v{h}", name=f"v{h}")
                for h in range(HPC)]
        outT = [arrs.tile([P, BT], BF16, tag=f"o{h}", name=f"o{h}")
                for h in range(HPC)]
        vtmp = [arrs.tile([P, IC], BF16, tag=f"vt{h}", name=f"vt{h}")
                for h in range(HPC)]

        dests = [("q", 0), ("k", 0), ("q", 1), ("k", 1), ("v", 0), ("v", 1)]
        for tch in range(TCH):
            tsl = slice(tch * IC, (tch + 1) * IC)
            if tch == 0:
                xt = xt0
            else:
                xt = xpool.tile([P, KD, IC], BF16, tag="xt", name="xt")
                nc.sync.dma_start(
                    xt, xT[tch * P:(tch + 1) * P].rearrange(
                        "p (ko t) -> p ko t", ko=KD))
            for di, (nm, h) in enumerate(dests):
                ps = psum.tile([P, IC], F32, tag=f"pj{di}", name=f"pj{di}")
                for kt in range(KD):
                    nc.tensor.matmul(
                        ps, w_sb[nm][:, kt, h * D:(h + 1) * D], xt[:, kt],
                        start=(kt == 0), stop=(kt == KD - 1),
                    )
                if nm == "q":
                    nc.vector.tensor_copy(qT[h][:, tsl], ps)
                elif nm == "k":
                    nc.vector.tensor_copy(kT[h][:, tsl], ps)
                else:
                    nc.vector.tensor_copy(vtmp[h], ps)
            # transpose v chunk -> v_sb (token tiles on partitions)
            b = (tch * IC) // T
            jt0 = ((tch * IC) % T) // P
            for h in range(HPC):
                pst = psum.tile([P, IC], BF16, tag="tr", name="tr")
                for q4 in range(4):
                    nc.tensor.transpose(
                        pst[:, q4 * P:(q4 + 1) * P],
                        vtmp[h][:, q4 * P:(q4 + 1) * P], ident)
                nc.vector.tensor_copy(v_sb[h][:, b, jt0:jt0 + 4], pst)

        # ---- phase B: attention with out-projection units woven in ----
        # out-projection is decomposed into units (2 matmuls + 1 cast); a
        # FIFO pool feeds one unit per 2 j-tiles into the attention stream,
        # exactly the PE slack under the scalar exp pace. Leftovers drain
        # at the end with deep psum rotation.
        unit_pool = []
        pending_groups = []
        ustate = {"u": 0, "y_sb": None}

        def make_units(ic, b):
            t0 = (b * T + ic * IC) // P
            for tt in range(t0, t0 + IC // P):
                for mc in range(D_MODEL // IC):
                    unit_pool.append((tt, mc))

        def emit_unit(deep):
            tt, mc = unit_pool.pop(0)
            u = ustate["u"]; ustate["u"] += 1
            msl = slice(mc * IC, (mc + 1) * IC)
            if mc == 0:
                ustate["y_sb"] = ypool.tile([P, D_MODEL], BF16, tag="ysb",
                                            name="ysb")
            y_sb = ustate["y_sb"]
            ytag = ["tr", "pj0", "pj1", "pj2"][u % 4] if deep else "tr"
            ps_y = psum.tile([P, IC], F32, tag=ytag, name="yps")
            for h in range(HPC):
                nc.tensor.matmul(
                    ps_y, outT[h][:, tt * P:(tt + 1) * P], wo_sb[:, h, msl],
                    start=(h == 0), stop=(h == HPC - 1))
            if u % 2 == 0:
                nc.vector.tensor_copy(y_sb[:, msl], ps_y)
            else:
                nc.scalar.copy(y_sb[:, msl], ps_y)
            if mc == D_MODEL // IC - 1:
                half = D_MODEL // 2
                nc.sync.dma_start(y[tt * P:(tt + 1) * P, :half],
                                  y_sb[:, :half])
                nc.sync.dma_start(y[tt * P:(tt + 1) * P, half:],
                                  y_sb[:, half:])

        def attn_chunk(ck, ic, h, b):
            if pending_groups:
                make_units(*pending_groups.pop(0))
            i0 = b * T + ic * IC
            nj = 4 * (ic + 1)
            lo_of = lambda jt: max(jt - 4 * ic, 0) * P
            pt = ptpool.tile([P, 16 * IC], BF16, tag="pt", name="pt")

            def s_tile(jt):
                lo = lo_of(jt)
                ps_s = psum.tile([P, IC], F32, tag=f"pj{jt % 3}",
                                 name=f"pj{jt % 3}")
                nc.tensor.matmul(
                    ps_s[:, lo:],
                    kT[h][:, b * T + jt * P: b * T + (jt + 1) * P],
                    qT[h][:, i0 + lo: i0 + IC], start=True, stop=True)
                nc.scalar.activation(
                    pt[:, jt * IC + lo:(jt + 1) * IC], ps_s[:, lo:],
                    mybir.ActivationFunctionType.Exp, scale=scale)
                if jt - 4 * ic >= 0:
                    nc.vector.tensor_tensor(
                        pt[:, jt * IC + lo: jt * IC + lo + P],
                        pt[:, jt * IC + lo: jt * IC + lo + P],
                        tri_mask, mybir.AluOpType.mult)

            # denominator accumulates on DVE (bf16), freeing the PE
            pt_acc = smalls.tile([P, IC], BF16, tag="ptacc", name="ptacc")
            # PV double-buffered (pj3/pj4) so the norm chain of chunk n
            # overlaps chunk n+1's PV accumulation
            ps_o = psum.tile([P, IC], F32, tag=f"pj{3 + ck % 2}",
                             name=f"pj{3 + ck % 2}")

            # software pipeline: S runs one j-tile ahead of PV
            s_tile(0)
            for jt in range(nj):
                if jt + 1 < nj:
                    s_tile(jt + 1)
                lo = lo_of(jt)
                psl = slice(jt * IC + lo, (jt + 1) * IC)
                if jt == 0:
                    nc.vector.tensor_copy(pt_acc, pt[:, psl])
                else:
                    nc.vector.tensor_tensor(
                        pt_acc[:, lo:], pt_acc[:, lo:], pt[:, psl],
                        mybir.AluOpType.add)
                nc.tensor.matmul(
                    ps_o[:, lo:], v_sb[h][:, b, jt], pt[:, psl],
                    start=(jt == 0), stop=(jt == nj - 1),
                    skip_group_check=True)
                if jt % 2 == 1 and unit_pool:
                    emit_unit(deep=False)

            ps_d = psum.tile([P, IC], F32, tag="pj5", name="pj5")
            r = (ck % 3) * 32
            nc.tensor.matmul(ps_d[r:r + 1], ones_col, pt_acc,
                             start=True, stop=True, skip_group_check=True)
            den_sb = smalls.tile([1, IC], F32, tag="densb", name="densb")
            nc.vector.tensor_copy(den_sb, ps_d[r:r + 1])
            bc = smalls.tile([P, IC], F32, tag="bc", name="bc")
            nc.gpsimd.partition_broadcast(bc, den_sb)
            rb = smalls.tile([P, IC], F32, tag="rb", name="rb")
            nc.vector.reciprocal_approx_fast(out=rb, in_=bc)
            nc.vector.tensor_tensor(
                outT[h][:, i0:i0 + IC], ps_o, rb, mybir.AluOpType.mult)

        ck = 0
        for ic in range(NI):
            for h, b in ((0, 0), (1, 0), (0, 1), (1, 1)):
                attn_chunk(ck, ic, h, b); ck += 1
                if h == 1:
                    pending_groups.append((ic, b))
        for g in pending_groups:
            make_units(*g)
        while unit_pool:
            emit_unit(deep=True)


def _build():
    nc = bacc.Bacc("TRN2", target_bir_lowering=False, debug=False,
                   num_devices=NCORES)
    # host pre-tiles everything so each DMA is contiguous per partition
    xT = nc.dram_tensor("xT", [TCH * P, KD * IC], BF16,
                        kind="ExternalInput").ap()
    wqT = nc.dram_tensor("wqT", [P, KD * HPC * D], BF16,
                         kind="ExternalInput").ap()
    wkT = nc.dram_tensor("wkT", [P, KD * HPC * D], BF16,
                         kind="ExternalInput").ap()
    wvT = nc.dram_tensor("wvT", [P, KD * HPC * D], BF16,
                         kind="ExternalInput").ap()
    woT = nc.dram_tensor("woT", [P, HPC * D_MODEL], BF16,
                         kind="ExternalInput").ap()
    y = nc.dram_tensor("y", [BT, D_MODEL], BF16, kind="ExternalOutput").ap()
    with tile.TileContext(nc) as tc:
        _emit(tc, xT, wqT, wkT, wvT, woT, y)
    nc.compile()
    return nc


def _prep_inputs(x, Wq, Wk, Wv, Wo):
    bf = ml_dtypes.bfloat16

    def wtile(w):  # [D_MODEL, HPC*D] -> [P, KD*(HPC*D)], contiguous rows
        return np.ascontiguousarray(
            w.reshape(KD, P, HPC * D).transpose(1, 0, 2).reshape(P, -1)
        ).astype(bf)

    xT2 = np.asarray(x, np.float32).reshape(BT, D_MODEL).T  # [D_MODEL, BT]
    xT = np.ascontiguousarray(
        xT2.reshape(KD, P, TCH, IC).transpose(2, 1, 0, 3).reshape(TCH * P, -1)
    ).astype(bf)
    in_maps = []
    for c in range(NCORES):
        rows = slice(c * HPC * D, (c + 1) * HPC * D)
        woT2 = np.asarray(Wo)[:, rows].T  # [HPC*D, D_MODEL]
        in_maps.append({
            "xT": xT,
            "wqT": wtile(np.asarray(Wq)[rows].T),
            "wkT": wtile(np.asarray(Wk)[rows].T),
            "wvT": wtile(np.asarray(Wv)[rows].T),
            "woT": np.ascontiguousarray(
                woT2.reshape(HPC, P, D_MODEL).transpose(1, 0, 2)
                .reshape(P, -1)).astype(bf),
        })
    return in_maps


def run(x, Wq, Wk, Wv, Wo, cfg=None, trace=False):
    nc = _build()
    in_maps = _prep_inputs(x, Wq, Wk, Wv, Wo)
    try:
        res = run_bass_kernel_spmd(nc, in_maps, core_ids=list(range(NCORES)),
                                   trace=trace)
    except Exception:
        res = run_bass_kernel_spmd(nc, in_maps, core_ids=list(range(NCORES)),
                                   trace=trace)
    y = np.zeros((BT, D_MODEL), np.float32)
    for r in res.results:
        y += np.asarray(r["y"], dtype=np.float32)
    return y.reshape(B, T, D_MODEL), res


def kernel(x, Wq, Wk, Wv, Wo):
    y, _ = run(x, Wq, Wk, Wv, Wo)
    return y
